# revision 1
# baseline (speedup 1.0000x reference)
"""GQA attention layer for Trainium2, tensor-parallel over kv-heads on 8 NeuronCores.

Problem: x:(1,2048,2048) f32, causal mask; q/k/v/o projections with
NUM_HEADS=32, NUM_KV_HEADS=8, HEAD_DIM=128, GROUP=4.

Sharding: core c owns kv-head c and its 4 query heads (columns 4c*128..(4c+4)*128
of wq, rows of wo). Each core computes a partial y_c = attnout_c @ wo_c; the host
sums the 8 partials and adds bo.

Dataflow on each core (all "transposed" so no on-chip transposes of the big
probability matrix are ever needed):
  qT[d,i] = wq_c.T(h-major) accumulation:  matmul(lhsT=wq_kt, rhs=xT_kt)
  kT[d,j], vT[d,j] likewise;  v[j,d] via 16 PE transposes of vT.
  sT[j,i] = matmul(lhsT=kT_jtile, rhs=qT_chunk)   (contraction = head_dim, 1 mm)
  e = exp(sT * 1/sqrt(d))  on ACT (scale folded into qT drain), causal-masked
      with gpsimd.affine_select on the 4 diagonal j-tiles of each i-chunk.
  colsum[1,i] += ones.T @ e  (PE);  attnoutT[d,i] += v_jtile.T(=v) @ e (PE)
  recip = 1/colsum (DVE);  broadcast to 128 partitions with a k=1 PE matmul;
  aoT = avpsum * recip (DVE drain, bf16)
  y[i,hid] += aoT_head_tile.T @ wo_head  (4 head k-tiles), f32 out, DMA to HBM.

Causality: for i-chunk c (512 wide) only j-tiles 0..4c+3 are computed.
"""

import math

import numpy as np
import ml_dtypes

HIDDEN = 2048
HEAD_DIM = 128
NUM_HEADS = 32
NUM_KV = 8
GROUP = NUM_HEADS // NUM_KV
S = 2048
NCORES = 8
CH = 512                      # i-chunk width
NCH = S // CH                 # 4 i-chunks
KT = HIDDEN // 128            # 16 contraction tiles over hidden
NJT = S // 128                # 16 j-tiles
INV_SQRT_D = 1.0 / math.sqrt(HEAD_DIM)

# Module-level knobs for test.py (the grading harness uses the defaults).
TRACE = False
LAST_EXEC_NS = None
LAST_RESULTS = None

_PROG_CACHE = {}


def _build(mode):
    """mode: 'causal' (skip upper blocks, affine_select diag), 'full' (all-ones
    mask), 'generic' (multiplicative bf16 mask tiles from HBM)."""
    import concourse.bacc as bacc
    import concourse.tile as tile
    import concourse.mybir as mybir
    from concourse.masks import make_identity

    f32 = mybir.dt.float32
    bf16 = mybir.dt.bfloat16
    Ident = mybir.ActivationFunctionType.Identity
    Exp = mybir.ActivationFunctionType.Exp

    nc = bacc.Bacc(None, target_bir_lowering=False)

    xT_d = nc.dram_tensor("xT", [HIDDEN, S], bf16, kind="ExternalInput")
    wq_d = nc.dram_tensor("wq", [HIDDEN, GROUP * HEAD_DIM], bf16, kind="ExternalInput")
    wk_d = nc.dram_tensor("wk", [HIDDEN, HEAD_DIM], bf16, kind="ExternalInput")
    wv_d = nc.dram_tensor("wv", [HIDDEN, HEAD_DIM], bf16, kind="ExternalInput")
    wo_d = nc.dram_tensor("wo", [GROUP * HEAD_DIM, HIDDEN], bf16, kind="ExternalInput")
    bias_d = nc.dram_tensor("biasp", [128, 6], f32, kind="ExternalInput")
    if mode == "causal":
        ms_d = nc.dram_tensor("mstrip", [128, 896], bf16, kind="ExternalInput")
    if mode == "generic":
        mk_d = nc.dram_tensor("maskT", [S, S], bf16, kind="ExternalInput")
    y_d = nc.dram_tensor("y", [S, HIDDEN], f32, kind="ExternalOutput")

    def nblocks(c):
        return 4 * c + 4 if mode == "causal" else NJT

    with tile.TileContext(nc) as tc:
        with (
            tc.tile_pool(name="consts", bufs=1) as consts,
            tc.tile_pool(name="xw", bufs=1) as xw,
            tc.tile_pool(name="proj", bufs=1) as proj,
            tc.tile_pool(name="epool", bufs=6) as epool,
            tc.tile_pool(name="rpool", bufs=2) as rpool,
            tc.tile_pool(name="ypool", bufs=3) as ypool,
            tc.tile_pool(name="pp", bufs=2, space="PSUM") as pp,
            tc.tile_pool(name="spp", bufs=4, space="PSUM") as spp,
            tc.tile_pool(name="avp", bufs=1, space="PSUM") as avp,
            tc.tile_pool(name="csp", bufs=1, space="PSUM") as csp,
        ):
            # ---- constants ----
            ident = consts.tile([128, 128], bf16, tag="ident", name="ident")
            make_identity(nc, ident)
            ones_col = consts.tile([128, 1], bf16, tag="ones_col", name="ones_col")
            nc.vector.memset(ones_col, 1.0)
            f16 = mybir.dt.float16
            ones_row = consts.tile([1, 128], f16, tag="ones_row", name="ones_row")
            nc.vector.memset(ones_row, 1.0)
            bias_sb = consts.tile([128, 6], f32, tag="bias", name="bias_sb")
            nc.sync.dma_start(out=bias_sb, in_=bias_d[:, :])
            if mode == "causal":
                mstrip = consts.tile([128, 896], bf16, tag="mstrip", name="mstrip")
                nc.sync.dma_start(out=mstrip, in_=ms_d[:, :])

            # ---- input loads (k-tile-major so first proj block starts early) ----
            x_sb, wk_sb, wv_sb, wq_sb, wo_sb = [], [], [], [], []
            for kt in range(KT):
                wkt = xw.tile([128, HEAD_DIM], bf16, tag=f"wk{kt}", name=f"wk{kt}")
                nc.sync.dma_start(out=wkt, in_=wk_d[kt * 128:(kt + 1) * 128, :])
                wk_sb.append(wkt)
                xt = xw.tile([128, S], bf16, tag=f"x{kt}", name=f"x{kt}")
                nc.sync.dma_start(out=xt, in_=xT_d[kt * 128:(kt + 1) * 128, :])
                x_sb.append(xt)
            for kt in range(KT):
                wvt = xw.tile([128, HEAD_DIM], bf16, tag=f"wv{kt}", name=f"wv{kt}")
                nc.sync.dma_start(out=wvt, in_=wv_d[kt * 128:(kt + 1) * 128, :])
                wv_sb.append(wvt)
            for kt in range(KT):
                wqt = xw.tile([128, GROUP * HEAD_DIM], bf16, tag=f"wq{kt}", name=f"wq{kt}")
                nc.sync.dma_start(out=wqt, in_=wq_d[kt * 128:(kt + 1) * 128, :])
                wq_sb.append(wqt)
            for h in range(GROUP):
                wot = xw.tile([128, HIDDEN], bf16, tag=f"wo{h}", name=f"wo{h}")
                nc.sync.dma_start(out=wot, in_=wo_d[h * 128:(h + 1) * 128, :])
                wo_sb.append(wot)

            # ---- K projection: kT[d, j] per j-chunk ----
            kT_c = []
            for c in range(NCH):
                ps = pp.tile([128, CH], f32, tag="pp", name=f"psk{c}")
                for kt in range(KT):
                    nc.tensor.matmul(ps, lhsT=wk_sb[kt],
                                     rhs=x_sb[kt][:, c * CH:(c + 1) * CH],
                                     start=(kt == 0), stop=(kt == KT - 1))
                kt_t = proj.tile([128, CH], bf16, tag=f"kT{c}", name=f"kT{c}")
                nc.scalar.activation(kt_t, ps, Ident, bias=bias_sb[:, 4:5])
                kT_c.append(kt_t)

            # ---- V projection (vT) + PE transpose to v[j, d] ----
            vT_c = []
            for c in range(NCH):
                ps = pp.tile([128, CH], f32, tag="pp", name=f"psv{c}")
                for kt in range(KT):
                    nc.tensor.matmul(ps, lhsT=wv_sb[kt],
                                     rhs=x_sb[kt][:, c * CH:(c + 1) * CH],
                                     start=(kt == 0), stop=(kt == KT - 1))
                vt_t = proj.tile([128, CH], bf16, tag=f"vT{c}", name=f"vT{c}")
                nc.scalar.activation(vt_t, ps, Ident, bias=bias_sb[:, 5:6])
                vT_c.append(vt_t)
            v_sb = []
            for b in range(NJT):
                tp = spp.tile([128, 128], bf16, tag="s", name=f"tp{b}")
                nc.tensor.transpose(
                    tp, vT_c[b // 4][:, (b % 4) * 128:(b % 4 + 1) * 128], ident)
                vt = proj.tile([128, 128], bf16, tag=f"v{b}", name=f"v{b}")
                nc.vector.tensor_copy(vt, tp)
                v_sb.append(vt)

            # ---- Q projection: qT[(h, c)]; fold 1/sqrt(d) + scaled bias ----
            qT = {}
            for c in range(NCH):
                for h in range(GROUP):
                    ps = pp.tile([128, CH], f32, tag="pp", name=f"psq{h}_{c}")
                    for kt in range(KT):
                        nc.tensor.matmul(
                            ps, lhsT=wq_sb[kt][:, h * 128:(h + 1) * 128],
                            rhs=x_sb[kt][:, c * CH:(c + 1) * CH],
                            start=(kt == 0), stop=(kt == KT - 1))
                    qt_t = proj.tile([128, CH], bf16, tag=f"q{h}_{c}", name=f"q{h}_{c}")
                    nc.scalar.activation(qt_t, ps, Ident,
                                         bias=bias_sb[:, h:h + 1], scale=INV_SQRT_D)
                    qT[(h, c)] = qt_t

            # ---- generic-mode mask tiles (per chunk, shared across heads) ----
            mask_sb = {}

            # ---- attention + output projection, chunk-major ----
            aoT = {}
            for c in range(NCH):
                nb = nblocks(c)
                if mode == "generic":
                    for b in range(nb):
                        mt = proj.tile([128, CH], bf16, tag=f"m{b}", name=f"m{b}_{c}")
                        nc.sync.dma_start(
                            out=mt,
                            in_=mk_d[b * 128:(b + 1) * 128, c * CH:(c + 1) * CH])
                        mask_sb[b] = mt
                for h in range(GROUP):
                    av = avp.tile([128, CH], f32, tag="av", name=f"av{h}_{c}")
                    cs = csp.tile([1, CH], f32, tag="cs", name=f"cs{h}_{c}")
                    e_tiles = {}

                    def tail(b, nb=nb, h=h, c=c, av=av, cs=cs, e_tiles=e_tiles):
                        e = e_tiles.pop(b)
                        nc.tensor.matmul(cs, lhsT=ones_col, rhs=e,
                                         start=(b == 0), stop=(b == nb - 1),
                                         skip_group_check=True)
                        nc.tensor.matmul(av, lhsT=v_sb[b], rhs=e,
                                         start=(b == 0), stop=(b == nb - 1),
                                         skip_group_check=True)

                    LAG = 3
                    for b in range(nb):
                        sp_t = spp.tile([128, CH], f32, tag="s", name=f"s{h}_{c}_{b}")
                        nc.tensor.matmul(
                            sp_t, lhsT=kT_c[b // 4][:, (b % 4) * 128:(b % 4 + 1) * 128],
                            rhs=qT[(h, c)], start=True, stop=True)
                        e = epool.tile([128, CH], bf16, tag="e", name=f"e{h}_{c}_{b}")
                        nc.scalar.activation(e, sp_t, Exp)
                        if mode == "causal" and b >= 4 * c:
                            dd = b - 4 * c
                            st = 384 - 128 * dd
                            nc.vector.tensor_mul(e, e, mstrip[:, st:st + CH])
                        elif mode == "generic":
                            nc.vector.tensor_mul(e, e, mask_sb[b])
                        e_tiles[b] = e
                        if b >= LAG:
                            tail(b - LAG)
                    for b in range(max(nb - LAG, 0), nb):
                        tail(b)
                    recip = rpool.tile([1, CH], f32, tag="recip", name=f"rc{h}_{c}")
                    nc.vector.reciprocal_approx_fast(recip, cs)
                    recip16 = rpool.tile([1, CH], f16, tag="recip16",
                                         name=f"rc16{h}_{c}")
                    nc.vector.tensor_copy(recip16, recip)
                    rb_ps = spp.tile([128, CH], f32, tag="s", name=f"rbp{h}_{c}")
                    nc.tensor.matmul(rb_ps, lhsT=ones_row, rhs=recip16,
                                     start=True, stop=True)
                    rb = rpool.tile([128, CH], f32, tag="rb", name=f"rb{h}_{c}")
                    nc.vector.tensor_copy(rb, rb_ps)
                    ao = proj.tile([128, CH], bf16, tag=f"ao{h}_{c}", name=f"ao{h}_{c}")
                    nc.vector.tensor_mul(ao, av, rb)
                    aoT[(h, c)] = ao
                # y projection for this chunk
                for it in range(CH // 128):
                    for nh in range(NCH):
                        yp = pp.tile([128, CH], f32, tag="pp", name=f"yp{c}_{it}_{nh}")
                        for h in range(GROUP):
                            nc.tensor.matmul(
                                yp, lhsT=aoT[(h, c)][:, it * 128:(it + 1) * 128],
                                rhs=wo_sb[h][:, nh * CH:(nh + 1) * CH],
                                start=(h == 0), stop=(h == GROUP - 1))
                        ysb = ypool.tile([128, CH], f32, tag="y", name=f"y{c}_{it}_{nh}")
                        nc.vector.tensor_copy(ysb, yp)
                        nc.sync.dma_start(
                            out=y_d[c * CH + it * 128: c * CH + (it + 1) * 128,
                                    nh * CH:(nh + 1) * CH],
                            in_=ysb)
    nc.finalize()
    return nc


def _get_prog(mode):
    if mode not in _PROG_CACHE:
        _PROG_CACHE[mode] = _build(mode)
    return _PROG_CACHE[mode]


_LDW_PATCHED = False


def _enable_ldw_opt():
    """walrus is invoked with --enable-ldw-opt=false by default; weight
    double-buffering is what lets LDWEIGHTS overlap the previous matmul's
    drain, which this kernel's back-to-back matmul stream needs."""
    # Tried --enable-ldw-opt=true: walrus CoreV3GenImpl visitInstLdweights
    # crashes on this toolchain, so the flag stays off (LDWEIGHTS serializes
    # with matmuls; per-MM floor ~380ns at N=512).
    return


def kernel(x, mask, wq, bq, wk, bk, wv, bv, wo, bo):
    global LAST_EXEC_NS, LAST_RESULTS
    from concourse.bass_utils import run_bass_kernel_spmd

    _enable_ldw_opt()

    bf = ml_dtypes.bfloat16
    x = np.asarray(x, dtype=np.float32)
    mask = np.asarray(mask)
    wq = np.asarray(wq, dtype=np.float32)
    bq = np.asarray(bq, dtype=np.float32)
    wk = np.asarray(wk, dtype=np.float32)
    bk = np.asarray(bk, dtype=np.float32)
    wv = np.asarray(wv, dtype=np.float32)
    bv = np.asarray(bv, dtype=np.float32)
    wo = np.asarray(wo, dtype=np.float32)
    bo = np.asarray(bo, dtype=np.float32)

    m2 = mask[0, 0]
    if np.array_equal(m2 != 0, np.tril(np.ones((S, S), dtype=bool))):
        mode = "causal"
    elif np.all(m2 != 0):
        mode = "full"
    else:
        mode = "generic"

    xT = np.ascontiguousarray(x[0].T).astype(bf)
    if mode == "causal":
        g = np.arange(896)[None, :]
        p = np.arange(128)[:, None]
        mstrip = (g - p >= 384).astype(bf)
    in_maps = []
    for c in range(NCORES):
        qs = slice(4 * c * 128, (4 * c + 4) * 128)
        ks = slice(c * 128, (c + 1) * 128)
        biasp = np.zeros((128, 6), np.float32)
        biasp[:, 0:4] = (bq[qs] * INV_SQRT_D).reshape(4, 128).T
        biasp[:, 4] = bk[ks]
        biasp[:, 5] = bv[ks]
        im = {
            "xT": xT,
            "wq": np.ascontiguousarray(wq[:, qs]).astype(bf),
            "wk": np.ascontiguousarray(wk[:, ks]).astype(bf),
            "wv": np.ascontiguousarray(wv[:, ks]).astype(bf),
            "wo": np.ascontiguousarray(wo[qs, :]).astype(bf),
            "biasp": biasp,
        }
        if mode == "causal":
            im["mstrip"] = mstrip
        if mode == "generic":
            im["maskT"] = np.ascontiguousarray((m2 != 0).T).astype(bf)
        in_maps.append(im)

    nc = _get_prog(mode)
    res = run_bass_kernel_spmd(nc, in_maps, list(range(NCORES)), trace=TRACE)
    LAST_EXEC_NS = res.exec_time_ns
    LAST_RESULTS = res
    y = np.zeros((S, HIDDEN), np.float64)
    for r in res.results:
        y += r["y"].astype(np.float64)
    y = (y + bo.astype(np.float64)).astype(np.float32)
    return y[None]



# revision 5
# speedup vs baseline: 1.1333x; 1.1333x over previous
"""GQA attention layer for Trainium2, tensor-parallel over kv-heads on 8 NeuronCores.

Problem: x:(1,2048,2048) f32, causal mask; q/k/v/o projections with
NUM_HEADS=32, NUM_KV_HEADS=8, HEAD_DIM=128, GROUP=4.

Sharding: core c owns kv-head c and its 4 query heads (columns 4c*128..(4c+4)*128
of wq, rows of wo). Each core computes a partial y_c = attnout_c @ wo_c; the host
sums the 8 partials and adds bo.

Dataflow on each core (transposed layout, no transposes of the probability
matrix):
  Per-chunk pipeline P(c) -> A(c) -> Y(c) over 4 i-chunks of 512:
  P(c): qT/kT/vT projections for chunk c from column-chunk x tiles
        (x DMA'd chunk-column-major so P(0) starts ~1us in);
        v[j,d] via 4 PE transposes of vT.
  A(c): per head h: for j-tile b in 0..4c+3:
          sT[j,i] = matmul(lhsT=kT_tile, rhs=qT)       (1 big MM)
          e = exp(sT)  on ACT (1/sqrt(d) folded into qT drain), diagonal
              j-tiles masked by a DVE multiply with a precomputed strip.
          eSum accumulation for the softmax denominator runs OFF the PE:
              odd b -> DVE eSum_v += e; even b -> GpSimd eSum_g += e.
          avT[d,i] += v_b.T @ e  (PE, lagged behind exp by LAG tiles)
        colsum = ones.T @ eSum_v + ones.T @ eSum_g  (2 small PE MMs)
        recip on DVE; broadcast to 128 partitions with a k=1 PE matmul;
        aoT = avpsum * recip_bcast (DVE, bf16)
  Y(c): y[i,hid] += aoT_head_tile.T @ wo_head (4 head k-tiles), f32, DMA out.

The big-MM stream keeps PE at the 216ns/MM issue rate (LDWEIGHTS hidden by the
PE reorder window); the previous version's per-j-tile colsum matmuls (M=1) broke
that hiding and cost ~65us of PE time.

Causality: for i-chunk c (512 wide) only j-tiles 0..4c+3 are computed.
"""

import math

import numpy as np
import ml_dtypes

HIDDEN = 2048
HEAD_DIM = 128
NUM_HEADS = 32
NUM_KV = 8
GROUP = NUM_HEADS // NUM_KV
S = 2048
NCORES = 8
CH = 512                      # i-chunk width
NCH = S // CH                 # 4 i-chunks
KT = HIDDEN // 128            # 16 contraction tiles over hidden
NJT = S // 128                # 16 j-tiles
INV_SQRT_D = 1.0 / math.sqrt(HEAD_DIM)

# Module-level knobs for test.py (the grading harness uses the defaults).
TRACE = False
LAST_EXEC_NS = None
LAST_RESULTS = None

# tuning knobs
LAG = 3                 # j-tiles between exp and the AV matmul consuming it
ESUM_GP = True          # even-b eSum adds on GpSimd (else all on DVE)

_PROG_CACHE = {}


def _build(mode):
    """mode: 'causal' (skip upper blocks, strip-mask diag), 'full' (all-ones
    mask), 'generic' (multiplicative bf16 mask tiles from HBM)."""
    import concourse.bacc as bacc
    import concourse.tile as tile
    import concourse.mybir as mybir
    from concourse.masks import make_identity

    f32 = mybir.dt.float32
    bf16 = mybir.dt.bfloat16
    f16 = mybir.dt.float16
    Ident = mybir.ActivationFunctionType.Identity
    Exp = mybir.ActivationFunctionType.Exp
    Add = mybir.AluOpType.add
    Mult = mybir.AluOpType.mult

    nc = bacc.Bacc(None, target_bir_lowering=False)

    xT_d = nc.dram_tensor("xT", [HIDDEN, S], bf16, kind="ExternalInput")
    wq_d = nc.dram_tensor("wq", [HIDDEN, GROUP * HEAD_DIM], bf16, kind="ExternalInput")
    wk_d = nc.dram_tensor("wk", [HIDDEN, HEAD_DIM], bf16, kind="ExternalInput")
    wv_d = nc.dram_tensor("wv", [HIDDEN, HEAD_DIM], bf16, kind="ExternalInput")
    wo_d = nc.dram_tensor("wo", [GROUP * HEAD_DIM, HIDDEN], bf16, kind="ExternalInput")
    bias_d = nc.dram_tensor("biasp", [128, 6], f32, kind="ExternalInput")
    if mode == "causal":
        ms_d = nc.dram_tensor("mstrip", [128, 896], bf16, kind="ExternalInput")
    if mode == "generic":
        mk_d = nc.dram_tensor("maskT", [S, S], bf16, kind="ExternalInput")
    y_d = nc.dram_tensor("y", [S, HIDDEN], f32, kind="ExternalOutput")

    def nblocks(c):
        return 4 * c + 4 if mode == "causal" else NJT

    with tile.TileContext(nc) as tc:
        with (
            tc.tile_pool(name="consts", bufs=1) as consts,
            tc.tile_pool(name="xw", bufs=1) as xw,
            tc.tile_pool(name="proj", bufs=1) as proj,
            tc.tile_pool(name="epool", bufs=8) as epool,
            tc.tile_pool(name="esp", bufs=4) as esp,
            tc.tile_pool(name="rpool", bufs=2) as rpool,
            tc.tile_pool(name="ypool", bufs=4) as ypool,
            tc.tile_pool(name="pp", bufs=3, space="PSUM") as pp,
            tc.tile_pool(name="spp", bufs=2, space="PSUM") as spp,
            tc.tile_pool(name="avp", bufs=2, space="PSUM") as avp,
            tc.tile_pool(name="csp", bufs=1, space="PSUM") as csp,
        ):
            # ---- constants ----
            bias_sb = consts.tile([128, 6], f32, tag="bias", name="bias_sb")
            nc.sync.dma_start(out=bias_sb, in_=bias_d[:, :])
            if mode == "causal":
                mstrip = consts.tile([128, 896], bf16, tag="mstrip", name="mstrip")
                nc.sync.dma_start(out=mstrip, in_=ms_d[:, :])
            ident = consts.tile([128, 128], bf16, tag="ident", name="ident")
            make_identity(nc, ident)
            ones_col = consts.tile([128, 1], bf16, tag="ones_col", name="ones_col")
            nc.vector.memset(ones_col, 1.0)
            ones_row = consts.tile([1, 128], f16, tag="ones_row", name="ones_row")
            nc.vector.memset(ones_row, 1.0)

            # ---- input loads ----
            # weights k-tile-major interleaved with chunk-0 x columns so the
            # first Q-proj matmul can start ~1us in; then x columns 1..3, wo.
            wk_sb, wv_sb, wq_sb, wo_sb = [], [], [], []
            x_sb = {}
            for kt in range(KT):
                wqt = xw.tile([128, GROUP * HEAD_DIM], bf16, tag=f"wq{kt}", name=f"wq{kt}")
                nc.sync.dma_start(out=wqt, in_=wq_d[kt * 128:(kt + 1) * 128, :])
                wq_sb.append(wqt)
                wkt = xw.tile([128, HEAD_DIM], bf16, tag=f"wk{kt}", name=f"wk{kt}")
                nc.sync.dma_start(out=wkt, in_=wk_d[kt * 128:(kt + 1) * 128, :])
                wk_sb.append(wkt)
                wvt = xw.tile([128, HEAD_DIM], bf16, tag=f"wv{kt}", name=f"wv{kt}")
                nc.sync.dma_start(out=wvt, in_=wv_d[kt * 128:(kt + 1) * 128, :])
                wv_sb.append(wvt)
                xt = xw.tile([128, CH], bf16, tag=f"x{kt}_0", name=f"x{kt}_0")
                nc.sync.dma_start(out=xt, in_=xT_d[kt * 128:(kt + 1) * 128, 0:CH])
                x_sb[(kt, 0)] = xt
            for c in range(1, NCH):
                for kt in range(KT):
                    xt = xw.tile([128, CH], bf16, tag=f"x{kt}_{c}", name=f"x{kt}_{c}")
                    nc.sync.dma_start(
                        out=xt, in_=xT_d[kt * 128:(kt + 1) * 128, c * CH:(c + 1) * CH])
                    x_sb[(kt, c)] = xt
            for h in range(GROUP):
                wot = xw.tile([128, HIDDEN], bf16, tag=f"wo{h}", name=f"wo{h}")
                nc.sync.dma_start(out=wot, in_=wo_d[h * 128:(h + 1) * 128, :])
                wo_sb.append(wot)

            qT = {}
            kT_c = []
            v_sb = []
            aoT = {}
            mask_sb = {}

            def phase_P(c):
                # Q projection for chunk c (4 heads), then K, V, V-transposes
                for h in range(GROUP):
                    ps = pp.tile([128, CH], f32, tag="pp", name=f"psq{h}_{c}")
                    for kt in range(KT):
                        nc.tensor.matmul(
                            ps, lhsT=wq_sb[kt][:, h * 128:(h + 1) * 128],
                            rhs=x_sb[(kt, c)],
                            start=(kt == 0), stop=(kt == KT - 1))
                    qt_t = proj.tile([128, CH], bf16, tag=f"q{h}_{c}", name=f"q{h}_{c}")
                    nc.scalar.activation(qt_t, ps, Ident,
                                         bias=bias_sb[:, h:h + 1], scale=INV_SQRT_D)
                    qT[(h, c)] = qt_t
                ps = pp.tile([128, CH], f32, tag="pp", name=f"psk{c}")
                for kt in range(KT):
                    nc.tensor.matmul(ps, lhsT=wk_sb[kt], rhs=x_sb[(kt, c)],
                                     start=(kt == 0), stop=(kt == KT - 1))
                kt_t = proj.tile([128, CH], bf16, tag=f"kT{c}", name=f"kT{c}")
                nc.scalar.activation(kt_t, ps, Ident, bias=bias_sb[:, 4:5])
                kT_c.append(kt_t)
                ps = pp.tile([128, CH], f32, tag="pp", name=f"psv{c}")
                for kt in range(KT):
                    nc.tensor.matmul(ps, lhsT=wv_sb[kt], rhs=x_sb[(kt, c)],
                                     start=(kt == 0), stop=(kt == KT - 1))
                vt_t = proj.tile([128, CH], bf16, tag=f"vT{c}", name=f"vT{c}")
                nc.scalar.activation(vt_t, ps, Ident, bias=bias_sb[:, 5:6])
                for dd in range(4):
                    b = 4 * c + dd
                    tp = spp.tile([128, 128], bf16, tag="s", name=f"tp{b}")
                    nc.tensor.transpose(
                        tp, vt_t[:, dd * 128:(dd + 1) * 128], ident)
                    vt = proj.tile([128, 128], bf16, tag=f"v{b}", name=f"v{b}")
                    nc.any.tensor_copy(vt, tp)
                    v_sb.append(vt)

            def phase_A(c):
                nb = nblocks(c)
                if mode == "generic":
                    for b in range(nb):
                        if b not in mask_sb:
                            mt = proj.tile([128, S], bf16, tag=f"m{b}", name=f"m{b}")
                            mask_sb[b] = mt
                        nc.sync.dma_start(
                            out=mask_sb[b][:, c * CH:(c + 1) * CH],
                            in_=mk_d[b * 128:(b + 1) * 128, c * CH:(c + 1) * CH])
                for h in range(GROUP):
                    av = avp.tile([128, CH], f32, tag="av", name=f"av{h}_{c}")
                    esum_v = esp.tile([128, CH], bf16, tag="esv", name=f"esv{h}_{c}")
                    esum_g = esp.tile([128, CH], bf16, tag="esg", name=f"esg{h}_{c}")
                    e_tiles = {}
                    # state per accumulator: [pending_first_tile, started]
                    st_v, st_g = [None, False], [None, False]

                    def accum(b, e, h=h, c=c):
                        # softmax-denominator accumulation off the PE:
                        # odd b -> DVE, even b -> GpSimd (two partial sums)
                        use_gp = ESUM_GP and (b % 2 == 0)
                        eng = nc.gpsimd if use_gp else nc.vector
                        acc = esum_g if use_gp else esum_v
                        st = st_g if use_gp else st_v
                        if not st[1] and st[0] is None:
                            st[0] = e
                        elif not st[1]:
                            eng.tensor_tensor(acc, st[0], e, op=Add)
                            st[0], st[1] = None, True
                        else:
                            eng.tensor_tensor(acc, acc, e, op=Add)

                    def tail(b, nb=nb, h=h, c=c, av=av, e_tiles=e_tiles):
                        e = e_tiles.pop(b)
                        nc.tensor.matmul(av, lhsT=v_sb[b], rhs=e,
                                         start=(b == 0), stop=(b == nb - 1),
                                         skip_group_check=True)

                    for b in range(nb):
                        sp_t = spp.tile([128, CH], f32, tag="s", name=f"s{h}_{c}_{b}")
                        nc.tensor.matmul(
                            sp_t, lhsT=kT_c[b // 4][:, (b % 4) * 128:(b % 4 + 1) * 128],
                            rhs=qT[(h, c)], start=True, stop=True)
                        e = epool.tile([128, CH], bf16, tag="e", name=f"e{h}_{c}_{b}")
                        nc.scalar.activation(e, sp_t, Exp)
                        if mode == "causal" and b >= 4 * c:
                            dd = b - 4 * c
                            st = 384 - 128 * dd
                            nc.vector.tensor_tensor(e, e, mstrip[:, st:st + CH], op=Mult)
                        elif mode == "generic":
                            nc.vector.tensor_tensor(
                                e, e, mask_sb[b][:, c * CH:(c + 1) * CH], op=Mult)
                        accum(b, e)
                        e_tiles[b] = e
                        if b >= LAG:
                            tail(b - LAG)
                    for b in range(max(nb - LAG, 0), nb):
                        tail(b)
                    # combine the two eSum partials into the denominator
                    cs = csp.tile([1, CH], f32, tag="cs", name=f"cs{h}_{c}")
                    have_g = st_g[1] or st_g[0] is not None
                    # single-tile partials were never added into the
                    # accumulator: use the e tile directly
                    rhs_v = esum_v if st_v[1] else st_v[0]
                    nc.tensor.matmul(cs, lhsT=ones_col, rhs=rhs_v,
                                     start=True, stop=not have_g,
                                     skip_group_check=True)
                    if have_g:
                        rhs_g = esum_g if st_g[1] else st_g[0]
                        nc.tensor.matmul(cs, lhsT=ones_col, rhs=rhs_g,
                                         start=False, stop=True,
                                         skip_group_check=True)
                    recip = rpool.tile([1, CH], f32, tag="recip", name=f"rc{h}_{c}")
                    nc.vector.reciprocal_approx_fast(recip, cs)
                    recip16 = rpool.tile([1, CH], f16, tag="recip16",
                                         name=f"rc16{h}_{c}")
                    nc.vector.tensor_copy(recip16, recip)
                    rb_ps = spp.tile([128, CH], f32, tag="s", name=f"rbp{h}_{c}")
                    nc.tensor.matmul(rb_ps, lhsT=ones_row, rhs=recip16,
                                     start=True, stop=True)
                    rb = rpool.tile([128, CH], f32, tag="rb", name=f"rb{h}_{c}")
                    nc.any.tensor_copy(rb, rb_ps)
                    ao = proj.tile([128, CH], bf16, tag=f"ao{h}_{c}", name=f"ao{h}_{c}")
                    nc.vector.tensor_tensor(ao, av, rb, op=Mult)
                    aoT[(h, c)] = ao

            def phase_Y(c):
                for it in range(CH // 128):
                    for nh in range(NCH):
                        yp = pp.tile([128, CH], f32, tag="pp", name=f"yp{c}_{it}_{nh}")
                        for h in range(GROUP):
                            nc.tensor.matmul(
                                yp, lhsT=aoT[(h, c)][:, it * 128:(it + 1) * 128],
                                rhs=wo_sb[h][:, nh * CH:(nh + 1) * CH],
                                start=(h == 0), stop=(h == GROUP - 1))
                        ysb = ypool.tile([128, CH], f32, tag="y", name=f"y{c}_{it}_{nh}")
                        nc.any.tensor_copy(ysb, yp)
                        nc.sync.dma_start(
                            out=y_d[c * CH + it * 128: c * CH + (it + 1) * 128,
                                    nh * CH:(nh + 1) * CH],
                            in_=ysb)

            phase_P(0)
            phase_A(0)
            phase_P(1)
            phase_A(1)
            phase_Y(0)
            phase_P(2)
            phase_A(2)
            phase_Y(1)
            phase_P(3)
            phase_A(3)
            phase_Y(2)
            phase_Y(3)
    nc.finalize()
    return nc


def _get_prog(mode):
    if mode not in _PROG_CACHE:
        _PROG_CACHE[mode] = _build(mode)
    return _PROG_CACHE[mode]


def kernel(x, mask, wq, bq, wk, bk, wv, bv, wo, bo):
    global LAST_EXEC_NS, LAST_RESULTS
    from concourse.bass_utils import run_bass_kernel_spmd

    bf = ml_dtypes.bfloat16
    x = np.asarray(x, dtype=np.float32)
    mask = np.asarray(mask)
    wq = np.asarray(wq, dtype=np.float32)
    bq = np.asarray(bq, dtype=np.float32)
    wk = np.asarray(wk, dtype=np.float32)
    bk = np.asarray(bk, dtype=np.float32)
    wv = np.asarray(wv, dtype=np.float32)
    bv = np.asarray(bv, dtype=np.float32)
    wo = np.asarray(wo, dtype=np.float32)
    bo = np.asarray(bo, dtype=np.float32)

    m2 = mask[0, 0]
    if np.array_equal(m2 != 0, np.tril(np.ones((S, S), dtype=bool))):
        mode = "causal"
    elif np.all(m2 != 0):
        mode = "full"
    else:
        mode = "generic"

    xT = np.ascontiguousarray(x[0].T).astype(bf)
    if mode == "causal":
        g = np.arange(896)[None, :]
        p = np.arange(128)[:, None]
        mstrip = (g - p >= 384).astype(bf)
    in_maps = []
    for c in range(NCORES):
        qs = slice(4 * c * 128, (4 * c + 4) * 128)
        ks = slice(c * 128, (c + 1) * 128)
        biasp = np.zeros((128, 6), np.float32)
        biasp[:, 0:4] = (bq[qs] * INV_SQRT_D).reshape(4, 128).T
        biasp[:, 4] = bk[ks]
        biasp[:, 5] = bv[ks]
        im = {
            "xT": xT,
            "wq": np.ascontiguousarray(wq[:, qs]).astype(bf),
            "wk": np.ascontiguousarray(wk[:, ks]).astype(bf),
            "wv": np.ascontiguousarray(wv[:, ks]).astype(bf),
            "wo": np.ascontiguousarray(wo[qs, :]).astype(bf),
            "biasp": biasp,
        }
        if mode == "causal":
            im["mstrip"] = mstrip
        if mode == "generic":
            im["maskT"] = np.ascontiguousarray((m2 != 0).T).astype(bf)
        in_maps.append(im)

    nc = _get_prog(mode)
    res = run_bass_kernel_spmd(nc, in_maps, list(range(NCORES)), trace=TRACE)
    LAST_EXEC_NS = res.exec_time_ns
    LAST_RESULTS = res
    y = np.zeros((S, HIDDEN), np.float64)
    for r in res.results:
        y += r["y"].astype(np.float64)
    y = (y + bo.astype(np.float64)).astype(np.float32)
    return y[None]


# revision 9
# speedup vs baseline: 1.2603x; 1.1121x over previous
"""GQA attention layer for Trainium2, tensor-parallel over kv-heads on 8 NeuronCores.

Problem: x:(1,2048,2048) f32, causal mask; q/k/v/o projections with
NUM_HEADS=32, NUM_KV_HEADS=8, HEAD_DIM=128, GROUP=4.

Sharding: core c owns kv-head c and its 4 query heads (columns 4c*128..(4c+4)*128
of wq, rows of wo). Each core computes a partial y_c = attnout_c @ wo_c; the host
sums the 8 partials and adds bo.

Dataflow on each core (transposed layout, no transposes of the probability
matrix). Per-chunk pipeline P(c) -> A(c) -> Y(c) over 4 i-chunks of 512:
  P(c): qT/kT/vT projections for chunk c; v[j,d] via 4 PE transposes of vT.
        Inputs arrive as a handful of large host-relayouted DMAs (the DMA
        issue path costs ~600ns of sequencer time per descriptor set, so
        many small DMAs serialize the start of the kernel).
  A(c): per head h: for j-tile b in 0..4c+3 (off = left columns of the
        i-chunk that are fully causally masked for this j-tile):
          sT[j,i] = matmul(lhsT=kT_tile, rhs=qT[:, off:])   (1 big MM)
          e = exp(sT) on ACT (1/sqrt(d) folded into qT bias), diagonal
              j-tiles masked by a strip multiply.
          softmax denominator: COLSUM='ve': DVE accumulates eSum += e
              in-place (masks go to GpSimd), one ones.T @ eSum PE matmul
              per head; COLSUM='pe': per-head batch of ones.T @ e_b PE
              matmuls at head end (masks on DVE).
          avT[d,i] += v_b.T @ e  (PE, lagged behind exp by LAG tiles)
        recip on DVE; broadcast to 128 partitions with a k=1 PE matmul;
        aoT = avpsum * recip_bcast (DVE, bf16)
  Y(c): y[i,hid] += aoT_head_tile.T @ wo_head (4 head k-tiles), f32;
        drains go PSUM->SBUF on whichever of ACT/DVE is free (nc.any),
        staged into [128,2048] rows so each output DMA is one large block.

The big-MM stream keeps PE at the 216ns/MM issue rate (LDWEIGHTS hidden by the
PE reorder window); per-j-tile M=1 colsum matmuls inside the stream would break
that hiding (~400ns extra per occurrence), which is why the denominator is
accumulated off the PE (or batched per head).

Causality: for i-chunk c (512 wide) only j-tiles 0..4c+3 are computed, and
within the 4 diagonal j-tiles the fully-masked left 128*dd columns are skipped.
"""

import math

import numpy as np
import ml_dtypes

HIDDEN = 2048
HEAD_DIM = 128
NUM_HEADS = 32
NUM_KV = 8
GROUP = NUM_HEADS // NUM_KV
S = 2048
NCORES = 8
CH = 512                      # i-chunk width
NCH = S // CH                 # 4 i-chunks
KT = HIDDEN // 128            # 16 contraction tiles over hidden
NJT = S // 128                # 16 j-tiles
INV_SQRT_D = 1.0 / math.sqrt(HEAD_DIM)

# Module-level knobs for test.py (the grading harness uses the defaults).
TRACE = False
LAST_EXEC_NS = None
LAST_RESULTS = None

# tuning knobs
LAG = 3                 # j-tiles between exp and the AV matmul consuming it
COLSUM = "ve"           # 've': DVE eSum accumulator; 'pe': batched PE matmuls

_PROG_CACHE = {}


def _build(mode):
    """mode: 'causal' (skip upper blocks, strip-mask diag), 'full' (all-ones
    mask), 'generic' (multiplicative bf16 mask tiles from HBM)."""
    import concourse.bacc as bacc
    import concourse.tile as tile
    import concourse.mybir as mybir
    from concourse.masks import make_identity

    f32 = mybir.dt.float32
    bf16 = mybir.dt.bfloat16
    f16 = mybir.dt.float16
    Ident = mybir.ActivationFunctionType.Identity
    Exp = mybir.ActivationFunctionType.Exp
    Add = mybir.AluOpType.add
    Mult = mybir.AluOpType.mult

    nc = bacc.Bacc(None, target_bir_lowering=False)

    # host-relayouted inputs: x as 4 chunk-column blocks [128, KT*CH],
    # weights k-tile-major in the free dim, so each is one large DMA.
    x_d = [nc.dram_tensor(f"xc{c}", [128, KT * CH], bf16, kind="ExternalInput")
           for c in range(NCH)]
    wq_d = nc.dram_tensor("wq", [128, KT * GROUP * HEAD_DIM], bf16, kind="ExternalInput")
    wk_d = nc.dram_tensor("wk", [128, KT * HEAD_DIM], bf16, kind="ExternalInput")
    wv_d = nc.dram_tensor("wv", [128, KT * HEAD_DIM], bf16, kind="ExternalInput")
    wo_d = nc.dram_tensor("wo", [128, GROUP * HIDDEN], bf16, kind="ExternalInput")
    bias_d = nc.dram_tensor("biasp", [128, 6], f32, kind="ExternalInput")
    if mode == "causal":
        ms_d = nc.dram_tensor("mstrip", [128, 896], bf16, kind="ExternalInput")
    if mode == "generic":
        mk_d = nc.dram_tensor("maskT", [S, S], bf16, kind="ExternalInput")
    y_d = nc.dram_tensor("y", [S, HIDDEN], f32, kind="ExternalOutput")

    def nblocks(c):
        return 4 * c + 4 if mode == "causal" else NJT

    with tile.TileContext(nc) as tc:
        with (
            tc.tile_pool(name="consts", bufs=1) as consts,
            tc.tile_pool(name="xw", bufs=1) as xw,
            tc.tile_pool(name="proj", bufs=1) as proj,
            tc.tile_pool(name="epool", bufs=(20 if COLSUM == "pe" else 8)) as epool,
            tc.tile_pool(name="esp", bufs=2) as esp,
            tc.tile_pool(name="rpool", bufs=2) as rpool,
            tc.tile_pool(name="ypool", bufs=2) as ypool,
            tc.tile_pool(name="pp", bufs=3, space="PSUM") as pp,
            tc.tile_pool(name="spp", bufs=2, space="PSUM") as spp,
            tc.tile_pool(name="avp", bufs=2, space="PSUM") as avp,
            tc.tile_pool(name="csp", bufs=1, space="PSUM") as csp,
        ):
            # ---- constants ----
            bias_sb = consts.tile([128, 6], f32, tag="bias", name="bias_sb")
            nc.sync.dma_start(out=bias_sb, in_=bias_d[:, :])
            if mode == "causal":
                mstrip = consts.tile([128, 896], bf16, tag="mstrip", name="mstrip")
                nc.sync.dma_start(out=mstrip, in_=ms_d[:, :])
            ident = consts.tile([128, 128], bf16, tag="ident", name="ident")
            make_identity(nc, ident)
            ones_col = consts.tile([128, 1], bf16, tag="ones_col", name="ones_col")
            nc.vector.memset(ones_col, 1.0)
            ones_row = consts.tile([1, 128], f16, tag="ones_row", name="ones_row")
            nc.vector.memset(ones_row, 1.0)

            # ---- input loads: few large DMAs, interleaved so the first
            # projection matmuls can start early ----
            wq_sb = xw.tile([128, KT * GROUP * HEAD_DIM], bf16, tag="wq", name="wq_sb")
            wk_sb = xw.tile([128, KT * HEAD_DIM], bf16, tag="wk", name="wk_sb")
            wv_sb = xw.tile([128, KT * HEAD_DIM], bf16, tag="wv", name="wv_sb")
            wo_sb = xw.tile([128, GROUP * HIDDEN], bf16, tag="wo", name="wo_sb")
            x_sb = [xw.tile([128, KT * CH], bf16, tag=f"xc{c}", name=f"xc{c}")
                    for c in range(NCH)]
            QW = KT * GROUP * HEAD_DIM // 4
            XW = KT * CH // 4
            for g in range(4):  # interleave wq / x-chunk-0 quarters
                nc.sync.dma_start(out=wq_sb[:, g * QW:(g + 1) * QW],
                                  in_=wq_d[:, g * QW:(g + 1) * QW])
                nc.sync.dma_start(out=x_sb[0][:, g * XW:(g + 1) * XW],
                                  in_=x_d[0][:, g * XW:(g + 1) * XW])
            nc.sync.dma_start(out=wk_sb, in_=wk_d[:, :])
            nc.sync.dma_start(out=wv_sb, in_=wv_d[:, :])
            for c in range(1, NCH):
                nc.sync.dma_start(out=x_sb[c], in_=x_d[c][:, :])
            nc.sync.dma_start(out=wo_sb, in_=wo_d[:, :])

            def xs(kt, c):
                return x_sb[c][:, kt * CH:(kt + 1) * CH]

            qT = {}
            kT_c = []
            v_sb = []
            aoT = {}
            mask_sb = {}

            def phase_P(c):
                # Q projection for chunk c (4 heads), then K, V, V-transposes
                for h in range(GROUP):
                    ps = pp.tile([128, CH], f32, tag="pp", name=f"psq{h}_{c}")
                    for kt in range(KT):
                        nc.tensor.matmul(
                            ps,
                            lhsT=wq_sb[:, kt * 512 + h * 128:kt * 512 + (h + 1) * 128],
                            rhs=xs(kt, c),
                            start=(kt == 0), stop=(kt == KT - 1))
                    qt_t = proj.tile([128, CH], bf16, tag=f"q{h}_{c}", name=f"q{h}_{c}")
                    nc.scalar.activation(qt_t, ps, Ident,
                                         bias=bias_sb[:, h:h + 1], scale=INV_SQRT_D)
                    qT[(h, c)] = qt_t
                ps = pp.tile([128, CH], f32, tag="pp", name=f"psk{c}")
                for kt in range(KT):
                    nc.tensor.matmul(ps, lhsT=wk_sb[:, kt * 128:(kt + 1) * 128],
                                     rhs=xs(kt, c),
                                     start=(kt == 0), stop=(kt == KT - 1))
                kt_t = proj.tile([128, CH], bf16, tag=f"kT{c}", name=f"kT{c}")
                nc.scalar.activation(kt_t, ps, Ident, bias=bias_sb[:, 4:5])
                kT_c.append(kt_t)
                ps = pp.tile([128, CH], f32, tag="pp", name=f"psv{c}")
                for kt in range(KT):
                    nc.tensor.matmul(ps, lhsT=wv_sb[:, kt * 128:(kt + 1) * 128],
                                     rhs=xs(kt, c),
                                     start=(kt == 0), stop=(kt == KT - 1))
                vt_t = proj.tile([128, CH], bf16, tag=f"vT{c}", name=f"vT{c}")
                nc.scalar.activation(vt_t, ps, Ident, bias=bias_sb[:, 5:6])
                for dd in range(4):
                    b = 4 * c + dd
                    tp = spp.tile([128, 128], bf16, tag="s", name=f"tp{b}")
                    nc.tensor.transpose(
                        tp, vt_t[:, dd * 128:(dd + 1) * 128], ident)
                    vt = proj.tile([128, 128], bf16, tag=f"v{b}", name=f"v{b}")
                    nc.vector.tensor_copy(vt, tp)
                    v_sb.append(vt)

            def phase_A(c):
                nb = nblocks(c)
                if mode == "generic":
                    for b in range(nb):
                        if b not in mask_sb:
                            mask_sb[b] = proj.tile([128, CH], bf16, tag=f"m{b}",
                                                   name=f"m{b}")
                        nc.sync.dma_start(
                            out=mask_sb[b],
                            in_=mk_d[b * 128:(b + 1) * 128, c * CH:(c + 1) * CH])

                def off_of(b):
                    if mode == "causal" and b >= 4 * c:
                        return 128 * (b - 4 * c)
                    return 0

                for h in range(GROUP):
                    av = avp.tile([128, CH], f32, tag="av", name=f"av{h}_{c}")
                    esum = esp.tile([128, CH], bf16, tag="es", name=f"es{h}_{c}")
                    e_tiles = {}
                    kept = []  # (b, off, e) for COLSUM='pe'

                    def tail(b, nb=nb, av=av, e_tiles=e_tiles):
                        off, e = e_tiles.pop(b)
                        nc.tensor.matmul(av[:, off:], lhsT=v_sb[b], rhs=e[:, off:],
                                         start=(b == 0), stop=(b == nb - 1),
                                         skip_group_check=True)

                    for b in range(nb):
                        off = off_of(b)
                        w = CH - off
                        sp_t = spp.tile([128, CH], f32, tag="s", name=f"s{h}_{c}_{b}")
                        nc.tensor.matmul(
                            sp_t[:, off:],
                            lhsT=kT_c[b // 4][:, (b % 4) * 128:(b % 4 + 1) * 128],
                            rhs=qT[(h, c)][:, off:], start=True, stop=True)
                        e = epool.tile([128, CH], bf16, tag="e", name=f"e{h}_{c}_{b}")
                        nc.scalar.activation(e[:, off:], sp_t[:, off:], Exp)
                        if mode == "causal" and b >= 4 * c:
                            meng = nc.gpsimd if COLSUM == "ve" else nc.vector
                            meng.tensor_tensor(e[:, off:], e[:, off:],
                                               mstrip[:, 384:384 + w], op=Mult)
                        elif mode == "generic":
                            nc.vector.tensor_tensor(e, e, mask_sb[b], op=Mult)
                        if COLSUM == "ve":
                            if b == 0:
                                nc.vector.tensor_copy(esum, e)
                            else:
                                nc.vector.tensor_tensor(
                                    esum[:, off:], esum[:, off:], e[:, off:], op=Add)
                        else:
                            kept.append((b, off, e))
                        e_tiles[b] = (off, e)
                        if b >= LAG:
                            tail(b - LAG)
                    for b in range(max(nb - LAG, 0), nb):
                        tail(b)
                    cs = csp.tile([1, CH], f32, tag="cs", name=f"cs{h}_{c}")
                    if COLSUM == "ve":
                        nc.tensor.matmul(cs, lhsT=ones_col, rhs=esum,
                                         start=True, stop=True,
                                         skip_group_check=True)
                    else:
                        # batched per-head colsum over the kept e tiles
                        for b, off, e in kept:
                            nc.tensor.matmul(cs[:, off:], lhsT=ones_col,
                                             rhs=e[:, off:],
                                             start=(b == 0), stop=(b == nb - 1),
                                             skip_group_check=True)
                    recip = rpool.tile([1, CH], f32, tag="recip", name=f"rc{h}_{c}")
                    nc.vector.reciprocal_approx_fast(recip, cs)
                    recip16 = rpool.tile([1, CH], f16, tag="recip16",
                                         name=f"rc16{h}_{c}")
                    nc.vector.tensor_copy(recip16, recip)
                    rb_ps = spp.tile([128, CH], f32, tag="s", name=f"rbp{h}_{c}")
                    nc.tensor.matmul(rb_ps, lhsT=ones_row, rhs=recip16,
                                     start=True, stop=True)
                    rb = rpool.tile([128, CH], f32, tag="rb", name=f"rb{h}_{c}")
                    nc.vector.tensor_copy(rb, rb_ps)
                    ao = proj.tile([128, CH], bf16, tag=f"ao{h}_{c}", name=f"ao{h}_{c}")
                    nc.vector.tensor_tensor(ao, av, rb, op=Mult)
                    aoT[(h, c)] = ao

            def phase_Y(c):
                for it in range(CH // 128):
                    ysb = ypool.tile([128, HIDDEN], f32, tag="y", name=f"y{c}_{it}")
                    for nh in range(NCH):
                        yp = pp.tile([128, CH], f32, tag="pp", name=f"yp{c}_{it}_{nh}")
                        for h in range(GROUP):
                            nc.tensor.matmul(
                                yp, lhsT=aoT[(h, c)][:, it * 128:(it + 1) * 128],
                                rhs=wo_sb[:, h * HIDDEN + nh * CH:
                                          h * HIDDEN + (nh + 1) * CH],
                                start=(h == 0), stop=(h == GROUP - 1))
                        nc.any.tensor_copy(ysb[:, nh * CH:(nh + 1) * CH], yp)
                    nc.sync.dma_start(
                        out=y_d[c * CH + it * 128: c * CH + (it + 1) * 128, :],
                        in_=ysb)

            phase_P(0)
            phase_A(0)
            phase_P(1)
            phase_A(1)
            phase_Y(0)
            phase_P(2)
            phase_A(2)
            phase_Y(1)
            phase_P(3)
            phase_A(3)
            phase_Y(2)
            phase_Y(3)
    nc.finalize()
    return nc


def _get_prog(mode):
    if mode not in _PROG_CACHE:
        _PROG_CACHE[mode] = _build(mode)
    return _PROG_CACHE[mode]


def kernel(x, mask, wq, bq, wk, bk, wv, bv, wo, bo):
    global LAST_EXEC_NS, LAST_RESULTS
    from concourse.bass_utils import run_bass_kernel_spmd

    bf = ml_dtypes.bfloat16
    x = np.asarray(x, dtype=np.float32)
    mask = np.asarray(mask)
    wq = np.asarray(wq, dtype=np.float32)
    bq = np.asarray(bq, dtype=np.float32)
    wk = np.asarray(wk, dtype=np.float32)
    bk = np.asarray(bk, dtype=np.float32)
    wv = np.asarray(wv, dtype=np.float32)
    bv = np.asarray(bv, dtype=np.float32)
    wo = np.asarray(wo, dtype=np.float32)
    bo = np.asarray(bo, dtype=np.float32)

    m2 = mask[0, 0]
    if np.array_equal(m2 != 0, np.tril(np.ones((S, S), dtype=bool))):
        mode = "causal"
    elif np.all(m2 != 0):
        mode = "full"
    else:
        mode = "generic"

    # x relayout: xc[c][p, kt*CH + j] = x[0][c*CH + j, kt*128 + p]
    xT = np.ascontiguousarray(x[0].T).astype(bf)          # [H, S]
    xr = xT.reshape(KT, 128, NCH, CH).transpose(2, 1, 0, 3)  # [c, p, kt, j]
    xcs = [np.ascontiguousarray(xr[c].reshape(128, KT * CH)) for c in range(NCH)]
    if mode == "causal":
        g = np.arange(896)[None, :]
        p = np.arange(128)[:, None]
        mstrip = (g - p >= 384).astype(bf)
    in_maps = []
    for core in range(NCORES):
        qs = slice(4 * core * 128, (4 * core + 4) * 128)
        ks = slice(core * 128, (core + 1) * 128)
        biasp = np.zeros((128, 6), np.float32)
        biasp[:, 0:4] = (bq[qs] * INV_SQRT_D).reshape(4, 128).T
        biasp[:, 4] = bk[ks]
        biasp[:, 5] = bv[ks]
        wq_c = wq[:, qs].astype(bf)            # [H, 512]
        wq_r = np.ascontiguousarray(
            wq_c.reshape(KT, 128, GROUP * HEAD_DIM).transpose(1, 0, 2)
            .reshape(128, KT * GROUP * HEAD_DIM))
        wk_r = np.ascontiguousarray(
            wk[:, ks].astype(bf).reshape(KT, 128, HEAD_DIM).transpose(1, 0, 2)
            .reshape(128, KT * HEAD_DIM))
        wv_r = np.ascontiguousarray(
            wv[:, ks].astype(bf).reshape(KT, 128, HEAD_DIM).transpose(1, 0, 2)
            .reshape(128, KT * HEAD_DIM))
        wo_r = np.ascontiguousarray(
            wo[qs, :].astype(bf).reshape(GROUP, 128, HIDDEN).transpose(1, 0, 2)
            .reshape(128, GROUP * HIDDEN))
        im = {
            "wq": wq_r, "wk": wk_r, "wv": wv_r, "wo": wo_r, "biasp": biasp,
        }
        for c in range(NCH):
            im[f"xc{c}"] = xcs[c]
        if mode == "causal":
            im["mstrip"] = mstrip
        if mode == "generic":
            im["maskT"] = np.ascontiguousarray((m2 != 0).T).astype(bf)
        in_maps.append(im)

    nc = _get_prog(mode)
    res = run_bass_kernel_spmd(nc, in_maps, list(range(NCORES)), trace=TRACE)
    LAST_EXEC_NS = res.exec_time_ns
    LAST_RESULTS = res
    y = np.zeros((S, HIDDEN), np.float64)
    for r in res.results:
        y += r["y"].astype(np.float64)
    y = (y + bo.astype(np.float64)).astype(np.float32)
    return y[None]
